# revision 1
# baseline (speedup 1.0000x reference)
"""Fused transformer block (self-attn + cross-attn + MLP, post-LN) on TRN2.

Data-parallel over batch B=8: core b computes batch element b, no collectives.

Layout strategy: activations are held feature-major ("transposed"):
  actT[p, ci, t] = act[t, ci*128 + p]   (C on partitions, tokens in free dim)
so every linear contraction over C has its contraction dim on partitions for
both operands.  Attention per head h uses Q.T/K.T slabs [128=d, T] directly
(head dim D == 128 == partitions).  Scores are computed transposed
(S.T[keys, q]) so softmax'd probabilities P.T feed the O = P@V matmul without
any transposes; the softmax denominator (a cross-partition sum) is computed
with a ones-matmul that also broadcasts it across all 128 partitions.
LayerNorm mean / mean-of-squares (feature reductions = partition reductions)
use the same ones-matmul trick; rstd = exp(-0.5*ln(var+eps)) keeps ACT in the
exp/ln table set.  Matmuls run as float32r (fp32 data, 1 cycle/row for
N>=256); attention internals (Q/K/V/P) are bf16, safe here because the
reference scales scores by 1/sqrt(seq_len), making logits tiny.
"""

import math

import numpy as np

import concourse.bass as bass
import concourse.tile as tile
from concourse import bacc, mybir
from concourse.bass_utils import run_bass_kernel_spmd
from concourse.masks import make_identity

F32 = mybir.dt.float32
F32R = mybir.dt.float32r
BF16 = mybir.dt.bfloat16
AF = mybir.ActivationFunctionType
ALU = mybir.AluOpType

P = 128
B = 8
C = 512
H = 4
HID = 2048
EPS = 1e-5
CK = C // P      # 4 feature chunks
MH = HID // P    # 16 hidden chunks

WEIGHT_NAMES = [
    "sa_Wq", "sa_bq", "sa_Wk", "sa_bk", "sa_Wv", "sa_bv", "sa_Wo", "sa_bo",
    "ca_Wq", "ca_bq", "ca_Wk", "ca_bk", "ca_Wv", "ca_bv", "ca_Wo", "ca_bo",
    "ln_g", "ln_b", "fc1_W", "fc1_b", "fc2_W", "fc2_b",
]


def r32(ap):
    return ap.bitcast(F32R)


def _pin_act_tables():
    """Restrict activation-table choice to natural_log_exp_and_others, which
    contains every function this kernel uses (exp, ln, relu, copy, identity,
    square) — so the compile pass emits ONE table load instead of thrashing
    between the exp and natural_log sets at every layernorm."""
    from concourse import bacc as _bacc
    import functools

    if getattr(_bacc.get_activation_tables, "_pinned", False):
        return
    inner = _bacc.get_activation_tables

    @functools.cache
    def patched(arch):
        tabs = inner(arch)
        keep = "natural_log_exp_and_others"
        if keep not in tabs:
            return tabs
        return {name: (funcs if name == keep else set())
                for name, funcs in tabs.items()}

    patched._pinned = True
    _bacc.get_activation_tables = patched


def build_nc(T=2048, repeat=1, ablate=()):
    """Build the per-core Bass program. T = sequence length of x and y.

    repeat > 1 re-emits the whole computation; slope timing
    (t[k] - t[1]) / (k - 1) cancels the axon dispatch overhead."""
    assert T % 1024 == 0
    _pin_act_tables()
    NT = T // P       # 128-token tiles
    NCH = T // 512    # 512-token chunks
    NQG = T // 1024   # 1024-token query groups
    scale = 1.0 / math.sqrt(T)

    nc = bacc.Bacc("TRN2", target_bir_lowering=False, debug=False,
                   enable_asserts=False, num_devices=B)

    x_d = nc.dram_tensor("x", (T, C), F32, kind="ExternalInput")
    y_d = nc.dram_tensor("y", (T, C), F32, kind="ExternalInput")
    wshape = {"fc1_W": (C, HID), "fc2_W": (HID, C), "fc1_b": (HID,)}
    wd = {}
    for nm in WEIGHT_NAMES:
        shp = wshape.get(nm, (C, C) if "W" in nm else (C,))
        wd[nm] = nc.dram_tensor(nm, shp, F32, kind="ExternalInput")
    out_d = nc.dram_tensor("out", (T, C), F32, kind="ExternalOutput")

    xr = x_d.ap().rearrange("(tg p) c -> p tg c", p=P)
    yr = y_d.ap().rearrange("(tg p) c -> p tg c", p=P)
    outr = out_d.ap().rearrange("(tg p) c -> p tg c", p=P)

    def bias_pp(ap):  # [K] dram -> [128, K/128] partition-major view
        return ap.rearrange("(mo p) -> p mo", p=P)

    def bcast_row(ap):  # [n] dram -> broadcast-over-partitions [128, n] AP
        return bass.AP(tensor=ap.tensor, offset=ap.offset, ap=[[0, P]] + list(ap.ap))

    with tile.TileContext(nc) as tc:
        with (
            tc.tile_pool(name="psA", bufs=2, space="PSUM") as psA,
            tc.tile_pool(name="psB", bufs=2, space="PSUM") as psB,
            tc.tile_pool(name="psC", bufs=2, space="PSUM") as psC,
            tc.tile_pool(name="glob", bufs=1) as glob,
            tc.tile_pool(name="big4", bufs=2) as big4,
            tc.tile_pool(name="sqp", bufs=4) as sqp,
            tc.tile_pool(name="stat", bufs=4) as stat,
        ):
            # ---------------- constants ----------------
            ident = glob.tile([P, P], F32)
            make_identity(nc, ident[:])
            ones_raw = glob.tile([P, P], F32)
            nc.vector.memset(ones_raw[:], 1.0)
            ones_f = glob.tile([P, P], F32)
            nc.scalar.copy(r32(ones_f[:]), ones_raw[:])
            ones_b = glob.tile([P, P], BF16)
            nc.vector.memset(ones_b[:], 1.0)
            eps_t = glob.tile([P, 1], F32)
            nc.vector.memset(eps_t[:], EPS)

            bt = {}
            for nm in ["sa_bq", "sa_bk", "sa_bo", "ca_bq", "ca_bk", "ca_bo",
                       "fc2_b", "ln_g", "ln_b"]:
                bt[nm] = glob.tile([P, CK], F32, tag=nm, name=nm)
                nc.gpsimd.dma_start(out=bt[nm][:], in_=bias_pp(wd[nm].ap()))
            bt["fc1_b"] = glob.tile([P, MH], F32, tag="fc1_b", name="fc1_b")
            nc.gpsimd.dma_start(out=bt["fc1_b"][:], in_=bias_pp(wd["fc1_b"].ap()))
            for nm in ["sa_bv", "ca_bv"]:
                bt[nm] = glob.tile([P, C], F32, tag=nm, name=nm)
                nc.gpsimd.dma_start(out=bt[nm][:], in_=bcast_row(wd[nm].ap()))

            # ---------------- helpers ----------------
            def ln_chunk(residT, r_off, outT, o_off, round_out=True):
                """512-token-chunk LayerNorm: outT[:, :, o_off+..] = LN(chunk)*g+b."""
                if "no_ln" in ablate:
                    for ci in range(CK):
                        oap = outT[:, ci, o_off:o_off + 512]
                        nc.vector.tensor_copy(
                            r32(oap) if round_out else oap,
                            residT[:, ci, r_off:r_off + 512])
                    return
                s1 = psB.tile([P, 512], F32, tag="psB")
                for ci in range(CK):
                    nc.tensor.matmul(s1[:], r32(ones_f[:]),
                                     r32(residT[:, ci, r_off:r_off + 512]),
                                     start=(ci == 0), stop=(ci == CK - 1))
                s2 = psC.tile([P, 512], F32, tag="psC")
                for ci in range(CK):
                    sq = sqp.tile([P, 512], F32, tag="sqp")
                    nc.gpsimd.tensor_mul(r32(sq[:]), residT[:, ci, r_off:r_off + 512],
                                         residT[:, ci, r_off:r_off + 512])
                    nc.tensor.matmul(s2[:], r32(ones_f[:]), r32(sq[:]),
                                     start=(ci == 0), stop=(ci == CK - 1))
                mean = stat.tile([P, 512], F32, tag="stat")
                nc.vector.tensor_scalar(mean[:], s1[:], 1.0 / C, None, ALU.mult)
                var = stat.tile([P, 512], F32, tag="stat")
                nc.vector.tensor_scalar(var[:], s2[:], 1.0 / C, None, ALU.mult)
                msq = stat.tile([P, 512], F32, tag="stat")
                nc.vector.tensor_tensor(msq[:], mean[:], mean[:], ALU.mult)
                nc.vector.tensor_tensor(var[:], var[:], msq[:], ALU.subtract)
                rstd = stat.tile([P, 512], F32, tag="stat")
                nc.scalar.activation(rstd[:], var[:], AF.Ln, bias=eps_t[:])
                nc.scalar.activation(rstd[:], rstd[:], AF.Exp, scale=-0.5)
                for ci in range(CK):
                    tmp = sqp.tile([P, 512], F32, tag="sqp")
                    nc.gpsimd.tensor_tensor(tmp[:], residT[:, ci, r_off:r_off + 512],
                                            mean[:], ALU.subtract)
                    oap = outT[:, ci, o_off:o_off + 512]
                    rap = r32(oap) if round_out else oap
                    nc.vector.tensor_tensor(rap, tmp[:], rstd[:], ALU.mult)
                    nc.gpsimd.tensor_scalar(
                        rap, oap,
                        bt["ln_g"][:, ci:ci + 1], bt["ln_b"][:, ci:ci + 1],
                        ALU.mult, ALU.add)

            def one_repeat():
                # ================= attention era =================
                with (
                    tc.tile_pool(name="wat", bufs=2) as wat,
                    tc.tile_pool(name="bfa", bufs=3) as bfa,
                    tc.tile_pool(name="oq", bufs=1) as oqp,
                    tc.tile_pool(name="tstage", bufs=2) as tstage,
                    tc.tile_pool(name="ptile", bufs=3) as ptilep,
                    tc.tile_pool(name="recp", bufs=2) as recp,
                ):
                    def wtile(nm):
                        t = wat.tile([P, CK, C], F32, tag="wat", name=nm + "_t")
                        nc.sync.dma_start(
                            out=r32(t[:]),
                            in_=r32(wd[nm].ap().rearrange("(kc p) n -> p kc n", p=P)))
                        return t

                    def transpose_chunk_in(src_r, ch, dstT, dst_off):
                        stg = tstage.tile([P, 4, C], F32, tag="tstage")
                        nc.sync.dma_start(out=stg[:],
                                          in_=src_r[:, ch * 4:(ch + 1) * 4, :])
                        for ci in range(CK):
                            ps = psB.tile([P, 512], F32, tag="psB")
                            for j in range(4):
                                nc.tensor.transpose(ps[:, j * P:(j + 1) * P],
                                                    stg[:, j, ci * P:(ci + 1) * P],
                                                    ident[:])
                            nc.vector.tensor_copy(
                                r32(dstT[:, ci, dst_off:dst_off + 512]), ps[:])

                    def linear_fmajor(w_t, srcT, dstT, bias):
                        for chg in range(NQG):
                            for mo in range(CK):
                                ps = psA.tile([P, 1024], F32, tag="psA")
                                for half in range(2):
                                    o = (chg * 2 + half) * 512
                                    for kc in range(CK):
                                        nc.tensor.matmul(
                                            ps[:, half * 512:(half + 1) * 512],
                                            r32(w_t[:, kc, mo * P:(mo + 1) * P]),
                                            r32(srcT[:, kc, o:o + 512]),
                                            start=(kc == 0), stop=(kc == CK - 1))
                                nc.vector.tensor_scalar(
                                    dstT[:, mo, chg * 1024:(chg + 1) * 1024], ps[:],
                                    bias[:, mo:mo + 1], None, ALU.add)

                    def v_proj_tile(w_t, srcT, dstV, bv_bc, to):
                        ps = psA.tile([P, 512], F32, tag="psA")
                        for kc in range(CK):
                            nc.tensor.matmul(
                                ps[:], r32(srcT[:, kc, to * P:(to + 1) * P]),
                                r32(w_t[:, kc, :]),
                                start=(kc == 0), stop=(kc == CK - 1))
                        nc.vector.tensor_tensor(dstV[:, to, :], ps[:], bv_bc[:],
                                                ALU.add)

                    def attention_qg(qT, kT, V, wo_t, bo, residT, qg,
                                     post_h=None):
                        if True:
                            q0 = qg * 1024
                            oqt = oqp.tile([P, CK, 1024], F32, tag="oq")
                            if "no_attn_core" in ablate:
                                for ci in range(CK):
                                    nc.vector.tensor_copy(
                                        r32(oqt[:, ci, :]),
                                        qT[:, ci, q0:q0 + 1024])
                            nH = H if "no_attn_core" not in ablate else 0
                            # flattened (h, kt) software pipeline: the next
                            # step's S-matmuls + exp are emitted before this
                            # step's O/den matmuls, across h boundaries, so
                            # the ACT exp stream never stalls.
                            steps = [(h, kt) for h in range(nH)
                                     for kt in range(NT)]
                            pts = [None, None]
                            accs = {}

                            def s_exp(idx):
                                h, kt = steps[idx]
                                sps = psA.tile([P, 1024], F32, tag="psA",
                                               name="sps")
                                for qc in range(2):
                                    nc.tensor.matmul(
                                        sps[:, qc * 512:(qc + 1) * 512],
                                        kT[:, h, kt * P:(kt + 1) * P],
                                        qT[:, h, q0 + qc * 512:q0 + (qc + 1) * 512],
                                        start=True, stop=True)
                                pt = ptilep.tile([P, 1024], BF16, tag="ptile",
                                                 name="pt")
                                nc.scalar.activation(pt[:], sps[:], AF.Exp,
                                                     scale=scale)
                                pts[idx % 2] = pt

                            if steps:
                                s_exp(0)
                            for idx in range(len(steps)):
                                h, kt = steps[idx]
                                if idx + 1 < len(steps):
                                    s_exp(idx + 1)
                                if kt == 0:
                                    ops = [psB.tile([P, 512], F32, tag="psB",
                                                    name="ops") for _ in range(2)]
                                    dps = ([psC.tile([P, 512], F32, tag="psC",
                                                     name="dps")
                                            for _ in range(2)]
                                           if "no_denom" not in ablate
                                           else [None, None])
                                    accs[h] = (ops, dps)
                                ops, dps = accs[h]
                                pt = pts[idx % 2]
                                for qc in range(2):
                                    nc.tensor.matmul(
                                        ops[qc][:], V[:, kt, h * P:(h + 1) * P],
                                        pt[:, qc * 512:(qc + 1) * 512],
                                        start=(kt == 0), stop=(kt == NT - 1))
                                    if "no_denom" not in ablate:
                                        nc.tensor.matmul(
                                            dps[qc][:], ones_b[:],
                                            pt[:, qc * 512:(qc + 1) * 512],
                                            start=(kt == 0), stop=(kt == NT - 1))
                                if kt == NT - 1:
                                    for qc in range(2):
                                        if "no_denom" in ablate:
                                            nc.vector.tensor_copy(
                                                r32(oqt[:, h,
                                                        qc * 512:(qc + 1) * 512]),
                                                ops[qc][:])
                                            continue
                                        rc = recp.tile([P, 512], F32, tag="recp")
                                        nc.vector.reciprocal(rc[:], dps[qc][:])
                                        nc.vector.tensor_tensor(
                                            r32(oqt[:, h, qc * 512:(qc + 1) * 512]),
                                            ops[qc][:], rc[:], ALU.mult)
                                    if post_h is not None:
                                        post_h(h)
                            for mo in range(CK):
                                ps = psA.tile([P, 1024], F32, tag="psA")
                                for half in range(2):
                                    for kc in range(CK):
                                        nc.tensor.matmul(
                                            ps[:, half * 512:(half + 1) * 512],
                                            r32(wo_t[:, kc, mo * P:(mo + 1) * P]),
                                            r32(oqt[:, kc,
                                                    half * 512:(half + 1) * 512]),
                                            start=(kc == 0), stop=(kc == CK - 1))
                                for half in range(2):
                                    o = q0 + half * 512
                                    xat = sqp.tile([P, 512], F32, tag="sqp")
                                    nc.vector.tensor_scalar(
                                        xat[:], ps[:, half * 512:(half + 1) * 512],
                                        bo[:, mo:mo + 1], 0.0, ALU.add, ALU.max)
                                    nc.vector.tensor_tensor(
                                        r32(residT[:, mo, o:o + 512]),
                                        residT[:, mo, o:o + 512], xat[:], ALU.add)

                    # ---- load + transpose x ----
                    xT = big4.tile([P, CK, T], F32, tag="big")
                    for ch in range(NCH):
                        transpose_chunk_in(xr, ch, xT, ch * 512)

                    # ---- SA projections ----
                    w = wtile("sa_Wq")
                    qT = bfa.tile([P, CK, T], BF16, tag="bfa")
                    linear_fmajor(w, xT, qT, bt["sa_bq"])
                    w = wtile("sa_Wk")
                    kT = bfa.tile([P, CK, T], BF16, tag="bfa")
                    linear_fmajor(w, xT, kT, bt["sa_bk"])
                    w = wtile("sa_Wv")
                    V = bfa.tile([P, NT, C], BF16, tag="bfa")
                    for to in range(NT):
                        v_proj_tile(w, xT, V, bt["sa_bv"], to)

                    # ---- SA attention, with CA K/V + LN1 + CA Q-proj
                    #      interleaved into its (ACT-bound) stream ----
                    sa_wo = wtile("sa_Wo")
                    osa = big4.tile([P, CK, T], F32, tag="big")
                    holder = {}
                    ca_wq = [None]

                    def ykv_chunk(ch):
                        kcT, Vc = holder["kcT"], holder["Vc"]
                        ca_wk, ca_wv = holder["ca_wk"], holder["ca_wv"]
                        yTc = tstage.tile([P, CK, 512], F32, tag="tstage",
                                          name="yTc")
                        stg = tstage.tile([P, 4, C], F32, tag="tstage", name="stg")
                        nc.sync.dma_start(out=stg[:],
                                          in_=yr[:, ch * 4:(ch + 1) * 4, :])
                        for ci in range(CK):
                            ps = psB.tile([P, 512], F32, tag="psB", name="ps")
                            for j in range(4):
                                nc.tensor.transpose(ps[:, j * P:(j + 1) * P],
                                                    stg[:, j, ci * P:(ci + 1) * P],
                                                    ident[:])
                            nc.vector.tensor_copy(r32(yTc[:, ci, :]), ps[:])
                        for mo in range(CK):
                            ps = psA.tile([P, 512], F32, tag="psA", name="ps")
                            for kc in range(CK):
                                nc.tensor.matmul(
                                    ps[:], r32(ca_wk[:, kc, mo * P:(mo + 1) * P]),
                                    r32(yTc[:, kc, :]),
                                    start=(kc == 0), stop=(kc == CK - 1))
                            nc.vector.tensor_scalar(
                                kcT[:, mo, ch * 512:(ch + 1) * 512],
                                ps[:], bt["ca_bk"][:, mo:mo + 1], None, ALU.add)
                        for j in range(4):
                            ps = psA.tile([P, 512], F32, tag="psA", name="ps")
                            for kc in range(CK):
                                nc.tensor.matmul(
                                    ps[:], r32(yTc[:, kc, j * P:(j + 1) * P]),
                                    r32(ca_wv[:, kc, :]),
                                    start=(kc == 0), stop=(kc == CK - 1))
                            nc.vector.tensor_tensor(Vc[:, ch * 4 + j, :], ps[:],
                                                    bt["ca_bv"][:], ALU.add)

                    def caq_mo(mo, chg):
                        qcT = holder["qcT"]
                        ps = psA.tile([P, 1024], F32, tag="psA", name="ps")
                        for half in range(2):
                            o = (chg * 2 + half) * 512
                            for kc in range(CK):
                                nc.tensor.matmul(
                                    ps[:, half * 512:(half + 1) * 512],
                                    r32(ca_wq[0][:, kc, mo * P:(mo + 1) * P]),
                                    r32(osa[:, kc, o:o + 512]),
                                    start=(kc == 0), stop=(kc == CK - 1))
                        nc.vector.tensor_scalar(
                            qcT[:, mo, chg * 1024:(chg + 1) * 1024], ps[:],
                            bt["ca_bq"][:, mo:mo + 1], None, ALU.add)

                    attention_qg(qT, kT, V, sa_wo, bt["sa_bo"], xT, 0)
                    for ch in (0, 1):
                        ln_chunk(xT, ch * 512, osa, ch * 512)
                    attention_qg(qT, kT, V, sa_wo, bt["sa_bo"], xT, 1)
                    for ch in (2, 3):
                        ln_chunk(xT, ch * 512, osa, ch * 512)
                    holder["ca_wk"] = wtile("ca_Wk")
                    holder["ca_wv"] = wtile("ca_Wv")
                    holder["kcT"] = bfa.tile([P, CK, T], BF16, tag="bfa",
                                             name="kcT")
                    holder["Vc"] = bfa.tile([P, NT, C], BF16, tag="bfa",
                                            name="Vc")
                    for ch in range(NCH):
                        ykv_chunk(ch)
                    ca_wq[0] = wtile("ca_Wq")
                    holder["qcT"] = bfa.tile([P, CK, T], BF16, tag="bfa",
                                             name="qcT")
                    for mo in range(CK):
                        caq_mo(mo, 0)
                        caq_mo(mo, 1)
                    kcT, Vc, qcT = holder["kcT"], holder["Vc"], holder["qcT"]

                    # ---- CA attention (+Wo, relu, resid into osa),
                    #      LN2 interleaved per query group ----
                    ca_wo = wtile("ca_Wo")
                    oca = big4.tile([P, CK, T], F32, tag="big")
                    for qg in range(NQG):
                        attention_qg(qcT, kcT, Vc, ca_wo, bt["ca_bo"], osa, qg)
                        for ch in range(2 * qg, 2 * qg + 2):
                            ln_chunk(osa, ch * 512, oca, ch * 512)

                # ================= MLP + LN3 + output =================
                with (
                    tc.tile_pool(name="wfc", bufs=2) as wfc,
                    tc.tile_pool(name="lno", bufs=1) as lno,
                    tc.tile_pool(name="otile", bufs=2) as otile,
                ):
                    w1 = wfc.tile([P, CK, HID], F32, tag="wfc")
                    w1r = wd["fc1_W"].ap().rearrange("(kc p) n -> p kc n", p=P)
                    for sp in range(4):
                        nc.sync.dma_start(
                            out=r32(w1[:, :, sp * 512:(sp + 1) * 512]),
                            in_=r32(w1r[:, :, sp * 512:(sp + 1) * 512]))
                    w2 = wfc.tile([P, MH, C], F32, tag="wfc")
                    w2r = wd["fc2_W"].ap().rearrange("(kc p) n -> p kc n", p=P)
                    for sp in range(4):
                        nc.sync.dma_start(
                            out=r32(w2[:, sp * 4:(sp + 1) * 4, :]),
                            in_=r32(w2r[:, sp * 4:(sp + 1) * 4, :]))

                    for ch in range(NCH):
                        o = ch * 512
                        if "no_mlp" in ablate:
                            lnt = lno.tile([P, CK, 512], F32, tag="lno")
                            ln_chunk(oca, o, lnt, 0, round_out=False)
                            for j in range(4):
                                ps = psB.tile([P, 512], F32, tag="psB")
                                for ci in range(CK):
                                    nc.tensor.transpose(
                                        ps[:, ci * P:(ci + 1) * P],
                                        lnt[:, ci, j * P:(j + 1) * P], ident[:])
                                ob = otile.tile([P, 512], F32, tag="otile")
                                nc.vector.tensor_copy(ob[:], ps[:])
                                nc.sync.dma_start(out=outr[:, ch * 4 + j, :],
                                                  in_=ob[:])
                            continue
                        hid = big4.tile([P, MH, 512], F32, tag="big")
                        for mo in range(MH):
                            ps = psA.tile([P, 512], F32, tag="psA")
                            for kc in range(CK):
                                nc.tensor.matmul(
                                    ps[:], r32(w1[:, kc, mo * P:(mo + 1) * P]),
                                    r32(oca[:, kc, o:o + 512]),
                                    start=(kc == 0), stop=(kc == CK - 1))
                            if mo % 2 == 0:
                                nc.scalar.activation(
                                    r32(hid[:, mo, :]), ps[:], AF.Relu,
                                    bias=bt["fc1_b"][:, mo:mo + 1])
                            else:
                                nc.vector.tensor_scalar(
                                    r32(hid[:, mo, :]), ps[:],
                                    bt["fc1_b"][:, mo:mo + 1], 0.0,
                                    ALU.add, ALU.max)
                        for mo in range(CK):
                            ps = psA.tile([P, 512], F32, tag="psA")
                            for kc in range(MH):
                                nc.tensor.matmul(
                                    ps[:], r32(w2[:, kc, mo * P:(mo + 1) * P]),
                                    r32(hid[:, kc, :]),
                                    start=(kc == 0), stop=(kc == MH - 1))
                            tmp = sqp.tile([P, 512], F32, tag="sqp")
                            nc.scalar.activation(tmp[:], ps[:], AF.Identity,
                                                 bias=bt["fc2_b"][:, mo:mo + 1])
                            nc.vector.tensor_tensor(r32(oca[:, mo, o:o + 512]),
                                                    tmp[:], oca[:, mo, o:o + 512],
                                                    ALU.add)
                        # LN3 chunk -> transpose to token-major -> DMA out
                        lnt = lno.tile([P, CK, 512], F32, tag="lno")
                        ln_chunk(oca, o, lnt, 0, round_out=False)
                        for j in range(4):
                            ps = psB.tile([P, 512], F32, tag="psB")
                            for ci in range(CK):
                                nc.tensor.transpose(ps[:, ci * P:(ci + 1) * P],
                                                    lnt[:, ci, j * P:(j + 1) * P],
                                                    ident[:])
                            ob = otile.tile([P, 512], F32, tag="otile")
                            nc.scalar.copy(ob[:], ps[:])
                            nc.sync.dma_start(out=outr[:, ch * 4 + j, :], in_=ob[:])

            for _rep in range(repeat):
                one_repeat()

    nc.compile()
    return nc


_CACHE = {}


def _compiled(T=2048, repeat=1, ablate=()):
    key = (T, repeat, tuple(ablate))
    if key not in _CACHE:
        _CACHE[key] = build_nc(T, repeat, ablate)
    return _CACHE[key]


def make_in_maps(inputs):
    inp = {k: np.ascontiguousarray(np.asarray(v, dtype=np.float32))
           for k, v in inputs.items()}
    return [
        {"x": inp["x"][b], "y": inp["y"][b],
         **{nm: inp[nm] for nm in WEIGHT_NAMES}}
        for b in range(B)
    ]


def kernel(**inputs):
    T = int(np.asarray(inputs["x"]).shape[1])
    nc = _compiled(T)
    in_maps = make_in_maps(inputs)
    res = run_bass_kernel_spmd(nc, in_maps, core_ids=list(range(B)))
    return np.stack([res.results[b]["out"] for b in range(B)], axis=0)


# ----------------------------------------------------------------------------
# Cached PJRT runner (adapted from bass2jax.run_bass_via_pjrt) so repeated
# calls reuse the jitted executable and device-resident inputs for timing.
# ----------------------------------------------------------------------------
class _Runner:
    def __init__(self, nc):
        import jax
        from jax.sharding import Mesh, PartitionSpec
        from jax.experimental.shard_map import shard_map
        from concourse import bass2jax as b2j
        from concourse import mybir as _mybir

        b2j.install_neuronx_cc_hook()
        self.jax = jax
        partition_name = (nc.partition_id_tensor.name
                          if nc.partition_id_tensor else None)
        self.partition_name = partition_name
        in_names, out_names, out_avals, zero_shapes = [], [], [], []
        for alloc in nc.m.functions[0].allocations:
            if not isinstance(alloc, _mybir.MemoryLocationSet):
                continue
            name = alloc.memorylocations[0].name
            if alloc.kind == "ExternalInput":
                if name != partition_name:
                    in_names.append(name)
            elif alloc.kind == "ExternalOutput":
                out_names.append(name)
                shape = tuple(alloc.tensor_shape)
                dtype = _mybir.dt.np(alloc.dtype)
                out_avals.append(jax.core.ShapedArray(shape, dtype))
                zero_shapes.append((shape, dtype))
        self.in_names, self.out_names = in_names, out_names
        n_params = len(in_names)
        n_outs = len(out_names)
        all_names = in_names + out_names
        if partition_name is not None:
            all_names = all_names + [partition_name]
        donate = tuple(range(n_params, n_params + n_outs))

        def _body(*args):
            operands = list(args)
            if partition_name is not None:
                operands.append(b2j.partition_id_tensor())
            outs = b2j._bass_exec_p.bind(
                *operands,
                out_avals=tuple(out_avals),
                in_names=tuple(all_names),
                out_names=tuple(out_names),
                lowering_input_output_aliases=(),
                sim_require_finite=True,
                sim_require_nnan=True,
                nc=nc,
            )
            return tuple(outs)

        devices = jax.devices()[:B]
        self.mesh = Mesh(np.asarray(devices), ("core",))
        in_specs = (PartitionSpec("core"),) * (n_params + n_outs)
        out_specs = (PartitionSpec("core"),) * n_outs
        self.fn = jax.jit(
            shard_map(_body, mesh=self.mesh, in_specs=in_specs,
                      out_specs=out_specs, check_rep=False),
            donate_argnums=donate, keep_unused=True)
        self.zero_shapes = zero_shapes
        self.out_avals = out_avals

    def put_inputs(self, in_maps):
        import jax
        from jax.sharding import NamedSharding, PartitionSpec
        sh = NamedSharding(self.mesh, PartitionSpec("core"))
        self.dev_in = [
            jax.device_put(
                np.concatenate([np.asarray(in_maps[c][nm]) for c in range(B)],
                               axis=0), sh)
            for nm in self.in_names
        ]

    def _zeros(self):
        import jax
        from jax.sharding import NamedSharding, PartitionSpec
        sh = NamedSharding(self.mesh, PartitionSpec("core"))
        return [jax.device_put(np.zeros((B * s[0], *s[1:]), d), sh)
                for (s, d) in self.zero_shapes]

    def run(self):
        return self.fn(*self.dev_in, *self._zeros())

    def run_np(self):
        outs = self.run()
        res = []
        for c in range(B):
            res.append({nm: np.asarray(outs[i]).reshape(
                B, *self.out_avals[i].shape)[c]
                for i, nm in enumerate(self.out_names)})
        return res


_RUNNERS = {}


def _runner(T=2048, repeat=1, ablate=()):
    key = (T, repeat, tuple(ablate))
    if key not in _RUNNERS:
        _RUNNERS[key] = _Runner(_compiled(T, repeat, ablate))
    return _RUNNERS[key]


def _median_time_ms(r, iters):
    import time
    outs = r.run()
    r.jax.block_until_ready(outs)
    times = []
    for _ in range(iters):
        zs = r._zeros()
        r.jax.block_until_ready(zs)
        t0 = time.perf_counter()
        outs = r.fn(*r.dev_in, *zs)
        r.jax.block_until_ready(outs)
        times.append((time.perf_counter() - t0) * 1e3)
    times.sort()
    return times


def time_kernel(inputs, iters=8, hi_repeat=4):
    """Slope-based device time: (t[hi_repeat] - t[1]) / (hi_repeat - 1), ns.

    Cancels the ~60-80 ms axon dispatch floor that hides the real kernel time.
    """
    T = int(np.asarray(inputs["x"]).shape[1])
    in_maps = make_in_maps(inputs)
    r1 = _runner(T, 1)
    r1.put_inputs(in_maps)
    t1 = _median_time_ms(r1, iters)
    rk = _runner(T, hi_repeat)
    rk.put_inputs(in_maps)
    tk = _median_time_ms(rk, iters)
    m1 = t1[len(t1) // 2]
    mk = tk[len(tk) // 2]
    per_iter_ms = (mk - m1) / (hi_repeat - 1)
    print(f"  [timing] repeat=1 median {m1:.2f} ms (min {t1[0]:.2f}); "
          f"repeat={hi_repeat} median {mk:.2f} ms (min {tk[0]:.2f}); "
          f"slope {per_iter_ms:.3f} ms/iter")
    return per_iter_ms * 1e6

